# revision 20
# baseline (speedup 1.0000x reference)
"""Trainium2 Bass kernel for the ConvFeatureExtractor problem.

Reference computation (all f32):
    matches[f, i] = sum_j kmer_params[f, kmer_idcs[i, j], j]      # (F, M)
    probs = softmax(matches / temperature, axis=1)                # over M
    pooled = freq @ probs.T                                       # (B, F)
    profile = pooled / pooled.sum(axis=1, keepdims=True)

Shapes: B=1024, M=4096 (=4^6 kmers), F=8192 filters, K=6, 4 bases.

Kernel strategy (8 NeuronCores, filter-sharded: FL = F/8 = 1024 per core):
  * Host folds 1/T, the per-(filter,position) max-shift and a x128 scale
    into params_eff; softmax is invariant to the shift, and the scale
    cancels in the final normalization.  E' = exp(matches_eff) lands in
    (0, 128] which fits fp8e4 (max 240) with all mass in normal range.
  * matches^T via K=24 matmuls, 4-row-packed into the PE array
    (tile_position row groups), exp on ScalarE written as fp8 E.
  * Z[f] = sum_i E[i, f] via DoubleRow ones-matmuls interleaved with
    phase A (broadcast across partitions for free).
  * U = freq @ E^T as fp8 DoubleRow matmuls (2 MACs/cell/cycle); batch
    tile 0 (and the Z matmuls) accumulate during phase A so the PE never
    idles while ScalarE drains exp.  Dep-free warm-up matmuls at the
    start and at the A->B seam keep the HAM clock gate at 8/8.
  * DVE applies 1/Z and row-reduces each batch tile straight from PSUM.
  * s AllReduce over the 8 cores split in two chunks (bt 0-2 early, the
    rest at the end) so the first collective's latency hides under the
    main GEMM and the second chains onto a warm ncfw stream.
Each core returns its (B, FL) f32 slice; host concatenates along F.
"""

import os

import numpy as np
import ml_dtypes

import concourse.bass as bass  # noqa: F401
import concourse.tile as tile
from concourse import bacc, mybir
from concourse.bass_utils import run_bass_kernel_spmd

NCORES = 8
B = 1024           # batch
M = 4096           # 4^6 kmers
F = 8192           # filters
KMER = 6           # kmer length
NBASE = 4
KK = NBASE * KMER  # 24 flattened (base, position)
FL = F // NCORES   # 1024 filters per core

MT = M // 128      # 32 contraction subtiles of 128
KP = MT // 2       # 16 DoubleRow k-pairs
NBT = B // 128     # 8 batch tiles
SPLIT_BT = 3       # batch tiles covered by the first (overlapped) AllReduce

BF16 = mybir.dt.bfloat16
FP8 = mybir.dt.float8e4
F32 = mybir.dt.float32
AFT = mybir.ActivationFunctionType
ALU = mybir.AluOpType
DR = mybir.MatmulPerfMode.DoubleRow

USE_FP8 = os.environ.get("KERNEL_BF16", "") in ("", "0")

_CACHE: dict = {}


def _body(tc, freqT, oh4, par4, out):
    nc = tc.nc
    stage = os.environ.get("KERNEL_STAGE", "")
    edt = FP8 if USE_FP8 else BF16
    with (
        tc.tile_pool(name="res", bufs=1) as res,
        tc.tile_pool(name="dram", bufs=1, space="DRAM") as dram,
    ):
        # ---------- constants / small inputs ----------
        oh_sb = res.tile([128, 8 * 128], BF16)      # 4-row-packed onehot^T
        nc.sync.dma_start(oh_sb[:], oh4[:])
        par_sb = res.tile([128, FL], BF16)          # 4-row-packed params_eff^T
        nc.sync.dma_start(par_sb[:], par4[:])
        ones_e = res.tile([128, 2, 128], edt)       # DoubleRow ones lhsT
        nc.vector.memset(ones_e[:], 1.0)
        # memset-initialized rhs for PE warm-up matmuls (no DMA dependency)
        warm_rhs = res.tile([128, 2, 512], edt)
        nc.vector.memset(warm_rhs[:], 1.0)

        # ---------- stream in freq^T (fp8/bf16, k-subtile major) ----------
        freq_sb = res.tile([128, MT, B], edt)
        for k in range(MT):
            nc.sync.dma_start(freq_sb[:, k, :], freqT[k * 128:(k + 1) * 128, :])

        E_sb = res.tile([128, MT, FL], edt)
        invz_bc = res.tile([128, FL], F32)
        pooled = res.tile([128, NBT * FL], F32)
        s_col = res.tile([128, NBT], F32)

        # single PSUM pool: pm 2x[128,2,512] + pu 2x[128,1024] = 8 banks
        with tc.tile_pool(name="ps", bufs=1, space="PSUM") as ps:
            zz = ps.tile([128, FL], F32, tag="pu", bufs=2)
            pu0 = ps.tile([128, FL], F32, tag="pu", bufs=2)

            # ---- PE warm-up: dep-free matmuls fill the input-DMA head so
            # the HAM clock gate reaches 8/8 before real compute starts.
            # They target zz with start/stop groups; the real Z accumulation
            # restarts the bank with start=True so the values are discarded.
            for i in range(12):
                if USE_FP8:
                    nc.tensor.matmul(zz[:, 0:512], lhsT=ones_e[:],
                                     rhs=warm_rhs[:], start=True, stop=True,
                                     perf_mode=DR)
                else:
                    nc.tensor.matmul(zz[:, 0:512], lhsT=ones_e[:, 0, :],
                                     rhs=warm_rhs[:, 0, :],
                                     start=True, stop=True)

            def bt_mms(bt, pu, kps, fcs):
                # main-GEMM contributions for batch tile bt, k-pairs kps
                for kp in kps:
                    for fc in fcs:
                        if USE_FP8:
                            nc.tensor.matmul(
                                pu[:, fc * 512:(fc + 1) * 512],
                                lhsT=freq_sb[:, 2 * kp:2 * kp + 2,
                                             bt * 128:(bt + 1) * 128],
                                rhs=E_sb[:, 2 * kp:2 * kp + 2,
                                         fc * 512:(fc + 1) * 512],
                                start=(kp == 0), stop=(kp == KP - 1),
                                perf_mode=DR)
                        else:
                            for k in (2 * kp, 2 * kp + 1):
                                nc.tensor.matmul(
                                    pu[:, fc * 512:(fc + 1) * 512],
                                    lhsT=freq_sb[:, k, bt * 128:(bt + 1) * 128],
                                    rhs=E_sb[:, k, fc * 512:(fc + 1) * 512],
                                    start=(k == 0), stop=(k == MT - 1))

            def z_mms(j, fcs):
                # Z accumulation for the pack produced in iteration j
                for kp, fc in ((kpj, fcj) for kpj in (2 * j, 2 * j + 1)
                               for fcj in fcs):
                    if USE_FP8:
                        nc.tensor.matmul(
                            zz[:, fc * 512:(fc + 1) * 512],
                            lhsT=ones_e[:],
                            rhs=E_sb[:, 2 * kp:2 * kp + 2,
                                     fc * 512:(fc + 1) * 512],
                            start=(kp == 0), stop=(kp == KP - 1),
                            perf_mode=DR)
                    else:
                        for k in (2 * kp, 2 * kp + 1):
                            nc.tensor.matmul(
                                zz[:, fc * 512:(fc + 1) * 512],
                                lhsT=ones_e[:, 0, :],
                                rhs=E_sb[:, k, fc * 512:(fc + 1) * 512],
                                start=(k == 0), stop=(k == MT - 1))

            for j in range(8):          # packs of 4 m-tiles (t = 4j+g)
                for fc in range(2):
                    # plug the early PE-idle hole so the HAM clock gate never
                    # sees a fully-idle window (only legal before the Z
                    # accumulation group opens in iteration (1, 0))
                    if (j, fc) in ((0, 0), (0, 1), (1, 0)):
                        for _ in range(5 if j == 0 else 3):
                            if USE_FP8:
                                nc.tensor.matmul(zz[:, 0:512],
                                                 lhsT=ones_e[:],
                                                 rhs=warm_rhs[:], start=True,
                                                 stop=True, perf_mode=DR)
                            else:
                                nc.tensor.matmul(zz[:, 0:512],
                                                 lhsT=ones_e[:, 0, :],
                                                 rhs=warm_rhs[:, 0, :],
                                                 start=True, stop=True)
                    pmA = ps.tile([128, 2, 512], F32, tag="pm", bufs=2,
                                   name=f"pmA{j}_{fc}")
                    pmB = ps.tile([128, 2, 512], F32, tag="pm", bufs=2,
                                   name=f"pmB{j}_{fc}")
                    for g in range(4):
                        pm = pmA if g < 2 else pmB
                        nc.tensor.matmul(
                            pm[:, g % 2, :],
                            lhsT=oh_sb[32 * g:32 * g + 32,
                                       j * 128:(j + 1) * 128],
                            rhs=par_sb[32 * g:32 * g + 32,
                                       fc * 512:(fc + 1) * 512],
                            start=True, stop=True,
                            tile_position=(32 * g, 0))
                    nc.scalar.activation(
                        E_sb[:, 4 * j:4 * j + 2, fc * 512:(fc + 1) * 512],
                        pmA[:], AFT.Exp)
                    nc.scalar.activation(
                        E_sb[:, 4 * j + 2:4 * j + 4, fc * 512:(fc + 1) * 512],
                        pmB[:], AFT.Exp)
                    if j > 0:
                        z_mms(j - 1, (fc,))
                        bt_mms(0, pu0, (2 * (j - 1), 2 * j - 1), (fc,))
            z_mms(7, (0, 1))
            bt_mms(0, pu0, (14, 15), (0, 1))
            # seam fill: keep the PE busy (and the HAM gate warm) while the
            # Z reciprocal frees the pu bank that batch tile 1 will reuse
            for _ in range(28):
                if USE_FP8:
                    nc.tensor.matmul(pmA[:, 0, :], lhsT=ones_e[:],
                                     rhs=warm_rhs[:], start=True, stop=True,
                                     perf_mode=DR)
                else:
                    nc.tensor.matmul(pmA[:, 0, :], lhsT=ones_e[:, 0, :],
                                     rhs=warm_rhs[:, 0, :],
                                     start=True, stop=True)
            nc.vector.reciprocal(invz_bc[:, 0:512], zz[:, 0:512])
            nc.vector.reciprocal(invz_bc[:, 512:1024], zz[:, 512:1024])

            if stage == "1":
                # bisect: dump E k-subtiles bt-shaped
                for bt in range(NBT):
                    prof = res.tile([128, FL], F32, tag="prof1", bufs=2,
                                    name=f"p1_{bt}")
                    nc.scalar.copy(prof[:], E_sb[:, 4 * bt, :])
                    nc.sync.dma_start(out[bt * 128:(bt + 1) * 128, :], prof[:])
                return

            # ------ phase B: U = freq @ E^T; 1/Z scale + rowsum ------
            s_sum = res.tile([128, NBT], F32)
            rinv = res.tile([128, NBT], F32)

            s_in1 = dram.tile([128, SPLIT_BT], F32)
            s_out1 = dram.tile([128, SPLIT_BT], F32, addr_space="Shared")
            s_in2 = dram.tile([128, NBT - SPLIT_BT], F32)
            s_out2 = dram.tile([128, NBT - SPLIT_BT], F32,
                               addr_space="Shared")

            no_cc = bool(os.environ.get("KERNEL_NO_COLLECTIVE"))

            def emit_collective(lo, hi, s_in, s_out):
                if no_cc:
                    nc.vector.tensor_scalar_mul(s_sum[:, lo:hi],
                                                s_col[:, lo:hi],
                                                float(NCORES))
                else:
                    nc.sync.dma_start(s_in[:], s_col[:, lo:hi])
                    nc.gpsimd.collective_compute(
                        "AllReduce", ALU.add,
                        replica_groups=[list(range(NCORES))],
                        ins=[s_in.opt()], outs=[s_out.opt()])
                    nc.sync.dma_start(s_sum[:, lo:hi], s_out[:])
                nc.vector.reciprocal(rinv[:, lo:hi], s_sum[:, lo:hi])

            s_half = res.tile([128, 2 * NBT], F32)

            def bt_half_tt(bt, pu, fc):
                # scale this psum half by 1/Z and row-reduce it; the fc=0
                # half runs while the fc=1 matmuls still accumulate
                sl = pooled[:, bt * FL + fc * 512:bt * FL + (fc + 1) * 512]
                nc.vector.tensor_mul(sl, pu[:, fc * 512:(fc + 1) * 512],
                                     invz_bc[:, fc * 512:(fc + 1) * 512])
                nc.vector.reduce_sum(s_half[:, 2 * bt + fc:2 * bt + fc + 1],
                                     sl, axis=mybir.AxisListType.X)

            def bt_finish(bt):
                nc.vector.tensor_add(s_col[:, bt:bt + 1],
                                     s_half[:, 2 * bt:2 * bt + 1],
                                     s_half[:, 2 * bt + 1:2 * bt + 2])
                if bt == SPLIT_BT - 1:
                    emit_collective(0, SPLIT_BT, s_in1, s_out1)

            def bt_epilogue(bt, pu):
                if stage == "2":
                    nc.scalar.copy(pooled[:, bt * FL:(bt + 1) * FL], pu[:])
                    nc.sync.dma_start(out[bt * 128:(bt + 1) * 128, :],
                                      pooled[:, bt * FL:(bt + 1) * FL])
                    return
                bt_half_tt(bt, pu, 0)
                bt_half_tt(bt, pu, 1)
                bt_finish(bt)

            bt_epilogue(0, pu0)
            for bt in range(1, NBT):
                pu = ps.tile([128, FL], F32, tag="pu", bufs=2,
                             name=f"pu{bt}")
                bt_mms(bt, pu, range(KP), (0, 1))
                bt_epilogue(bt, pu)
            if stage == "2":
                return
            emit_collective(SPLIT_BT, NBT, s_in2, s_out2)

            # ---------- profile = pooled * (1/s); write out ----------
            for bt in range(NBT):
                sl = pooled[:, bt * FL:(bt + 1) * FL]
                if stage == "3":
                    pass  # skip rinv scaling: dump pooled
                elif bt % 2 == 0:
                    nc.scalar.activation(sl, sl, AFT.Copy,
                                         scale=rinv[:, bt:bt + 1])
                else:
                    nc.vector.tensor_scalar_mul(sl, sl, rinv[:, bt:bt + 1])
                nc.sync.dma_start(out[bt * 128:(bt + 1) * 128, :], sl)


def _build_bass():
    nc = bacc.Bacc("TRN2", target_bir_lowering=False, debug=False,
                   num_devices=NCORES)
    idt = FP8 if USE_FP8 else BF16
    freqT = nc.dram_tensor("freqT", [M, B], idt, kind="ExternalInput").ap()
    oh4 = nc.dram_tensor("oh4", [128, 8 * 128], BF16, kind="ExternalInput").ap()
    par4 = nc.dram_tensor("par4", [128, FL], BF16, kind="ExternalInput").ap()
    out = nc.dram_tensor("out", [B, FL], F32, kind="ExternalOutput").ap()

    with tile.TileContext(nc) as tc:
        _body(tc, freqT, oh4, par4, out)
    nc.compile()
    return nc


def _get_nc():
    if "nc" not in _CACHE:
        _CACHE["nc"] = _build_bass()
    return _CACHE["nc"]


def _prepare_in_maps(freq, kmer_params, temperature, kmer_idcs):
    freq = np.asarray(freq, dtype=np.float32)            # (B, M)
    kp = np.asarray(kmer_params, dtype=np.float64)       # (F, 4, K)
    temp = float(np.asarray(temperature, dtype=np.float64).reshape(-1)[0])
    idcs = np.asarray(kmer_idcs).astype(np.int64)        # (M, K)

    assert freq.shape == (B, M) and kp.shape == (F, NBASE, KMER)
    assert idcs.shape == (M, KMER)

    # params_eff folds 1/T, the per-(f, j) max shift (softmax-invariant) and
    # ln(128)/K so that E' = exp(matches_eff) lies in (0, 128].
    shift = kp.max(axis=1) / temp                        # (F, K)
    scale_ln = np.log(128.0) / KMER if USE_FP8 else 0.0
    pf = (kp / temp - shift[:, None, :] + scale_ln)      # (F, 4, K)
    pf_flat = pf.reshape(F, KK).astype(np.float32)       # [f, c*K + j]

    # onehot^T of the kmer index input: ohT[c*K+j, i] = 1 iff idcs[i, j] == c
    onehot = np.zeros((M, NBASE, KMER), dtype=np.float32)
    onehot[np.arange(M)[:, None], idcs, np.arange(KMER)[None, :]] = 1.0
    ohT = onehot.reshape(M, KK).T                        # (24, M)

    # 4-row packing: row group g handles m-tiles t = 4j + g
    oh4 = np.zeros((128, 8, 128), dtype=np.float32)
    for g in range(NBASE):
        for j in range(8):
            t = 4 * j + g
            oh4[32 * g:32 * g + KK, j, :] = ohT[:, t * 128:(t + 1) * 128]
    oh4 = np.ascontiguousarray(
        oh4.reshape(128, 8 * 128)).astype(ml_dtypes.bfloat16)

    if USE_FP8:
        freqT = np.ascontiguousarray(freq.T * 128.0).astype(
            ml_dtypes.float8_e4m3)
    else:
        freqT = np.ascontiguousarray(freq.T).astype(ml_dtypes.bfloat16)

    in_maps = []
    for c in range(NCORES):
        pfc = pf_flat[c * FL:(c + 1) * FL]               # (FL, 24)
        par4 = np.zeros((128, FL), dtype=np.float32)
        for g in range(NBASE):
            par4[32 * g:32 * g + KK, :] = pfc.T
        in_maps.append({
            "freqT": freqT,
            "oh4": oh4,
            "par4": np.ascontiguousarray(par4).astype(ml_dtypes.bfloat16),
        })
    return in_maps


def _run(in_maps, trace=False):
    nc = _get_nc()
    return run_bass_kernel_spmd(nc, in_maps, list(range(NCORES)), trace=trace)


def kernel(freq, kmer_params, temperature, kmer_idcs):
    in_maps = _prepare_in_maps(freq, kmer_params, temperature, kmer_idcs)
    res = _run(in_maps,
               trace=os.environ.get("KERNEL_TRACE", "") not in ("", "0"))
    _CACHE["last_result"] = res
    return np.concatenate(
        [np.asarray(res.results[c]["out"], dtype=np.float32)
         for c in range(NCORES)], axis=1)


# revision 21
# speedup vs baseline: 1.2232x; 1.2232x over previous
"""Trainium2 Bass kernel for the ConvFeatureExtractor problem.

Reference computation (all f32):
    matches[f, i] = sum_j kmer_params[f, kmer_idcs[i, j], j]      # (F, M)
    probs = softmax(matches / temperature, axis=1)                # over M
    pooled = freq @ probs.T                                       # (B, F)
    profile = pooled / pooled.sum(axis=1, keepdims=True)

Shapes: B=1024, M=4096 (=4^6 kmers), F=8192 filters, K=6, 4 bases.

Kernel strategy (8 NeuronCores, filter-sharded: FL = F/8 = 1024 per core):
  * Host folds 1/T, the per-(filter,position) max-shift and a x128 scale
    into params_eff; softmax is invariant to the shift, and the scale
    cancels in the final normalization.  E' = exp(matches_eff) lands in
    (0, 128] which fits fp8e4 (max 240) with all mass in normal range.
  * matches^T via K=24 matmuls, 4-row-packed into the PE array
    (tile_position row groups), exp on ScalarE written as fp8 E.
  * Z[f] = sum_i E[i, f] via DoubleRow ones-matmuls interleaved with
    phase A (broadcast across partitions for free).
  * U = freq @ E^T as fp8 DoubleRow matmuls (2 MACs/cell/cycle); batch
    tile 0 (and the Z matmuls) accumulate during phase A so the PE never
    idles while ScalarE drains exp.  Dep-free warm-up matmuls at the
    start and at the A->B seam keep the HAM clock gate at 8/8.
  * DVE applies 1/Z and row-reduces each batch tile straight from PSUM.
  * s AllReduce over the 8 cores split in two chunks (bt 0-2 early, the
    rest at the end) so the first collective's latency hides under the
    main GEMM and the second chains onto a warm ncfw stream.
Each core returns its (B, FL) f32 slice; host concatenates along F.
"""

import os

import numpy as np
import ml_dtypes

import concourse.bass as bass  # noqa: F401
import concourse.tile as tile
from concourse import bacc, mybir
from concourse.bass_utils import run_bass_kernel_spmd

NCORES = 8
B = 1024           # batch
M = 4096           # 4^6 kmers
F = 8192           # filters
KMER = 6           # kmer length
NBASE = 4
KK = NBASE * KMER  # 24 flattened (base, position)
FL = F // NCORES   # 1024 filters per core

MT = M // 128      # 32 contraction subtiles of 128
KP = MT // 2       # 16 DoubleRow k-pairs
NBT = B // 128     # 8 batch tiles
SPLIT_BT = 3       # batch tiles covered by the first (overlapped) AllReduce

BF16 = mybir.dt.bfloat16
FP8 = mybir.dt.float8e4
F32 = mybir.dt.float32
AFT = mybir.ActivationFunctionType
ALU = mybir.AluOpType
DR = mybir.MatmulPerfMode.DoubleRow

USE_FP8 = os.environ.get("KERNEL_BF16", "") in ("", "0")

_CACHE: dict = {}


def _body(tc, freqT, oh4, par4, out):
    nc = tc.nc
    stage = os.environ.get("KERNEL_STAGE", "")
    edt = FP8 if USE_FP8 else BF16
    with (
        tc.tile_pool(name="res", bufs=1) as res,
        tc.tile_pool(name="dram", bufs=1, space="DRAM") as dram,
    ):
        # ---------- constants / small inputs ----------
        oh_sb = res.tile([128, 8 * 128], BF16)      # 4-row-packed onehot^T
        nc.sync.dma_start(oh_sb[:], oh4[:])
        par_sb = res.tile([128, FL], BF16)          # 4-row-packed params_eff^T
        nc.sync.dma_start(par_sb[:], par4[:])
        ones_e = res.tile([128, 2, 128], edt)       # DoubleRow ones lhsT
        nc.vector.memset(ones_e[:], 1.0)
        # memset-initialized rhs for PE warm-up matmuls (no DMA dependency)
        warm_rhs = res.tile([128, 2, 512], edt)
        nc.vector.memset(warm_rhs[:], 1.0)

        # ---------- stream in freq^T (fp8/bf16, k-subtile major) ----------
        freq_sb = res.tile([128, MT, B], edt)
        for k in range(MT):
            nc.sync.dma_start(freq_sb[:, k, :], freqT[k * 128:(k + 1) * 128, :])

        E_sb = res.tile([128, MT, FL], edt)
        invz_bc = res.tile([128, FL], F32)
        pooled = res.tile([128, NBT * FL], F32)
        s_col = res.tile([128, NBT], F32)

        # single PSUM pool: pm 2x[128,2,512] + pu 2x[128,1024] = 8 banks
        with tc.tile_pool(name="ps", bufs=1, space="PSUM") as ps:
            zz = ps.tile([128, FL], F32, tag="pu", bufs=2)
            pu0 = ps.tile([128, FL], F32, tag="pu", bufs=2)

            # ---- PE warm-up: dep-free matmuls fill the input-DMA head so
            # the HAM clock gate reaches 8/8 before real compute starts.
            # They target zz with start/stop groups; the real Z accumulation
            # restarts the bank with start=True so the values are discarded.
            for i in range(12):
                if USE_FP8:
                    nc.tensor.matmul(zz[:, 0:512], lhsT=ones_e[:],
                                     rhs=warm_rhs[:], start=True, stop=True,
                                     perf_mode=DR)
                else:
                    nc.tensor.matmul(zz[:, 0:512], lhsT=ones_e[:, 0, :],
                                     rhs=warm_rhs[:, 0, :],
                                     start=True, stop=True)

            def bt_mms(bt, pu, kps, fcs):
                # main-GEMM contributions for batch tile bt, k-pairs kps
                for kp in kps:
                    for fc in fcs:
                        if USE_FP8:
                            nc.tensor.matmul(
                                pu[:, fc * 512:(fc + 1) * 512],
                                lhsT=freq_sb[:, 2 * kp:2 * kp + 2,
                                             bt * 128:(bt + 1) * 128],
                                rhs=E_sb[:, 2 * kp:2 * kp + 2,
                                         fc * 512:(fc + 1) * 512],
                                start=(kp == 0), stop=(kp == KP - 1),
                                perf_mode=DR)
                        else:
                            for k in (2 * kp, 2 * kp + 1):
                                nc.tensor.matmul(
                                    pu[:, fc * 512:(fc + 1) * 512],
                                    lhsT=freq_sb[:, k, bt * 128:(bt + 1) * 128],
                                    rhs=E_sb[:, k, fc * 512:(fc + 1) * 512],
                                    start=(k == 0), stop=(k == MT - 1))

            def z_mms(j, fcs):
                # Z accumulation for the pack produced in iteration j
                for kp, fc in ((kpj, fcj) for kpj in (2 * j, 2 * j + 1)
                               for fcj in fcs):
                    if USE_FP8:
                        nc.tensor.matmul(
                            zz[:, fc * 512:(fc + 1) * 512],
                            lhsT=ones_e[:],
                            rhs=E_sb[:, 2 * kp:2 * kp + 2,
                                     fc * 512:(fc + 1) * 512],
                            start=(kp == 0), stop=(kp == KP - 1),
                            perf_mode=DR)
                    else:
                        for k in (2 * kp, 2 * kp + 1):
                            nc.tensor.matmul(
                                zz[:, fc * 512:(fc + 1) * 512],
                                lhsT=ones_e[:, 0, :],
                                rhs=E_sb[:, k, fc * 512:(fc + 1) * 512],
                                start=(k == 0), stop=(k == MT - 1))

            for j in range(8):          # packs of 4 m-tiles (t = 4j+g)
                for fc in range(2):
                    # plug the early PE-idle hole so the HAM clock gate never
                    # sees a fully-idle window (only legal before the Z
                    # accumulation group opens in iteration (1, 0))
                    if (j, fc) in ((0, 0), (0, 1), (1, 0)):
                        for _ in range(5 if j == 0 else 3):
                            if USE_FP8:
                                nc.tensor.matmul(zz[:, 0:512],
                                                 lhsT=ones_e[:],
                                                 rhs=warm_rhs[:], start=True,
                                                 stop=True, perf_mode=DR)
                            else:
                                nc.tensor.matmul(zz[:, 0:512],
                                                 lhsT=ones_e[:, 0, :],
                                                 rhs=warm_rhs[:, 0, :],
                                                 start=True, stop=True)
                    pmA = ps.tile([128, 2, 512], F32, tag="pm", bufs=2,
                                   name=f"pmA{j}_{fc}")
                    pmB = ps.tile([128, 2, 512], F32, tag="pm", bufs=2,
                                   name=f"pmB{j}_{fc}")
                    for g in range(4):
                        pm = pmA if g < 2 else pmB
                        nc.tensor.matmul(
                            pm[:, g % 2, :],
                            lhsT=oh_sb[32 * g:32 * g + 32,
                                       j * 128:(j + 1) * 128],
                            rhs=par_sb[32 * g:32 * g + 32,
                                       fc * 512:(fc + 1) * 512],
                            start=True, stop=True,
                            tile_position=(32 * g, 0))
                    nc.scalar.activation(
                        E_sb[:, 4 * j:4 * j + 2, fc * 512:(fc + 1) * 512],
                        pmA[:], AFT.Exp)
                    nc.scalar.activation(
                        E_sb[:, 4 * j + 2:4 * j + 4, fc * 512:(fc + 1) * 512],
                        pmB[:], AFT.Exp)
                    if j > 0:
                        z_mms(j - 1, (fc,))
                        bt_mms(0, pu0, (2 * (j - 1), 2 * j - 1), (fc,))
            z_mms(7, (0, 1))
            bt_mms(0, pu0, (14, 15), (0, 1))
            # seam fill: keep the PE busy (and the HAM gate warm) while the
            # Z reciprocal frees the pu bank that batch tile 1 will reuse
            for _ in range(28):
                if USE_FP8:
                    nc.tensor.matmul(pmA[:, 0, :], lhsT=ones_e[:],
                                     rhs=warm_rhs[:], start=True, stop=True,
                                     perf_mode=DR)
                else:
                    nc.tensor.matmul(pmA[:, 0, :], lhsT=ones_e[:, 0, :],
                                     rhs=warm_rhs[:, 0, :],
                                     start=True, stop=True)
            nc.vector.reciprocal(invz_bc[:, 0:512], zz[:, 0:512])
            nc.vector.reciprocal(invz_bc[:, 512:1024], zz[:, 512:1024])

            if stage == "1":
                # bisect: dump E k-subtiles bt-shaped
                for bt in range(NBT):
                    prof = res.tile([128, FL], F32, tag="prof1", bufs=2,
                                    name=f"p1_{bt}")
                    nc.scalar.copy(prof[:], E_sb[:, 4 * bt, :])
                    nc.sync.dma_start(out[bt * 128:(bt + 1) * 128, :], prof[:])
                return

            # ------ phase B: U = freq @ E^T; 1/Z scale + rowsum ------
            s_sum = res.tile([128, NBT], F32)
            rinv = res.tile([128, NBT], F32)

            s_in1 = dram.tile([128, SPLIT_BT], F32)
            s_out1 = dram.tile([128, SPLIT_BT], F32, addr_space="Shared")
            s_in2 = dram.tile([128, NBT - SPLIT_BT], F32)
            s_out2 = dram.tile([128, NBT - SPLIT_BT], F32,
                               addr_space="Shared")

            no_cc = bool(os.environ.get("KERNEL_NO_COLLECTIVE"))

            def emit_collective(lo, hi, s_in, s_out, skip_dma=0):
                if no_cc:
                    nc.vector.tensor_scalar_mul(s_sum[:, lo:hi],
                                                s_col[:, lo:hi],
                                                float(NCORES))
                else:
                    if skip_dma < hi - lo:
                        nc.sync.dma_start(s_in[:, skip_dma:],
                                          s_col[:, lo + skip_dma:hi])
                    nc.gpsimd.collective_compute(
                        "AllReduce", ALU.add,
                        replica_groups=[list(range(NCORES))],
                        ins=[s_in.opt()], outs=[s_out.opt()])
                    nc.sync.dma_start(s_sum[:, lo:hi], s_out[:])
                nc.vector.reciprocal(rinv[:, lo:hi], s_sum[:, lo:hi])

            s_half = res.tile([128, 2 * NBT], F32)

            def bt_half_tt(bt, pu, fc):
                # scale this psum half by 1/Z and row-reduce it; the fc=0
                # half runs while the fc=1 matmuls still accumulate
                sl = pooled[:, bt * FL + fc * 512:bt * FL + (fc + 1) * 512]
                nc.vector.tensor_mul(sl, pu[:, fc * 512:(fc + 1) * 512],
                                     invz_bc[:, fc * 512:(fc + 1) * 512])
                nc.vector.reduce_sum(s_half[:, 2 * bt + fc:2 * bt + fc + 1],
                                     sl, axis=mybir.AxisListType.X)

            def bt_finish(bt):
                nc.vector.tensor_add(s_col[:, bt:bt + 1],
                                     s_half[:, 2 * bt:2 * bt + 1],
                                     s_half[:, 2 * bt + 1:2 * bt + 2])
                if bt == SPLIT_BT - 1:
                    emit_collective(0, SPLIT_BT, s_in1, s_out1)
                if bt == NBT - 2 and not no_cc and stage != "2":
                    nc.sync.dma_start(s_in2[:, 0:NBT - 1 - SPLIT_BT],
                                      s_col[:, SPLIT_BT:NBT - 1])

            def bt_epilogue(bt, pu):
                if stage == "2":
                    nc.scalar.copy(pooled[:, bt * FL:(bt + 1) * FL], pu[:])
                    nc.sync.dma_start(out[bt * 128:(bt + 1) * 128, :],
                                      pooled[:, bt * FL:(bt + 1) * FL])
                    return
                bt_half_tt(bt, pu, 0)
                bt_half_tt(bt, pu, 1)
                bt_finish(bt)

            bt_epilogue(0, pu0)
            for bt in range(1, NBT):
                pu = ps.tile([128, FL], F32, tag="pu", bufs=2,
                             name=f"pu{bt}")
                if bt == NBT - 1 and stage != "2":
                    # last tile: finish the fc0 half two k-pairs early so its
                    # scale+reduce overlaps the final matmuls, shortening the
                    # chain from last-MM to the AllReduce trigger
                    bt_mms(bt, pu, range(KP - 2), (0, 1))
                    bt_mms(bt, pu, (KP - 2, KP - 1), (0,))
                    bt_half_tt(bt, pu, 0)
                    bt_mms(bt, pu, (KP - 2, KP - 1), (1,))
                    bt_half_tt(bt, pu, 1)
                    bt_finish(bt)
                else:
                    bt_mms(bt, pu, range(KP), (0, 1))
                    bt_epilogue(bt, pu)
            if stage == "2":
                return
            emit_collective(SPLIT_BT, NBT, s_in2, s_out2,
                            skip_dma=NBT - 1 - SPLIT_BT)

            # ---------- profile = pooled * (1/s); write out ----------
            for bt in range(NBT):
                sl = pooled[:, bt * FL:(bt + 1) * FL]
                if stage == "3":
                    pass  # skip rinv scaling: dump pooled
                elif bt % 2 == 0:
                    nc.scalar.activation(sl, sl, AFT.Copy,
                                         scale=rinv[:, bt:bt + 1])
                else:
                    nc.vector.tensor_scalar_mul(sl, sl, rinv[:, bt:bt + 1])
                nc.sync.dma_start(out[bt * 128:(bt + 1) * 128, :], sl)


def _build_bass():
    nc = bacc.Bacc("TRN2", target_bir_lowering=False, debug=False,
                   num_devices=NCORES)
    idt = FP8 if USE_FP8 else BF16
    freqT = nc.dram_tensor("freqT", [M, B], idt, kind="ExternalInput").ap()
    oh4 = nc.dram_tensor("oh4", [128, 8 * 128], BF16, kind="ExternalInput").ap()
    par4 = nc.dram_tensor("par4", [128, FL], BF16, kind="ExternalInput").ap()
    out = nc.dram_tensor("out", [B, FL], F32, kind="ExternalOutput").ap()

    with tile.TileContext(nc) as tc:
        _body(tc, freqT, oh4, par4, out)
    nc.compile()
    return nc


def _get_nc():
    if "nc" not in _CACHE:
        _CACHE["nc"] = _build_bass()
    return _CACHE["nc"]


def _prepare_in_maps(freq, kmer_params, temperature, kmer_idcs):
    freq = np.asarray(freq, dtype=np.float32)            # (B, M)
    kp = np.asarray(kmer_params, dtype=np.float64)       # (F, 4, K)
    temp = float(np.asarray(temperature, dtype=np.float64).reshape(-1)[0])
    idcs = np.asarray(kmer_idcs).astype(np.int64)        # (M, K)

    assert freq.shape == (B, M) and kp.shape == (F, NBASE, KMER)
    assert idcs.shape == (M, KMER)

    # params_eff folds 1/T, the per-(f, j) max shift (softmax-invariant) and
    # ln(128)/K so that E' = exp(matches_eff) lies in (0, 128].
    shift = kp.max(axis=1) / temp                        # (F, K)
    scale_ln = np.log(128.0) / KMER if USE_FP8 else 0.0
    pf = (kp / temp - shift[:, None, :] + scale_ln)      # (F, 4, K)
    pf_flat = pf.reshape(F, KK).astype(np.float32)       # [f, c*K + j]

    # onehot^T of the kmer index input: ohT[c*K+j, i] = 1 iff idcs[i, j] == c
    onehot = np.zeros((M, NBASE, KMER), dtype=np.float32)
    onehot[np.arange(M)[:, None], idcs, np.arange(KMER)[None, :]] = 1.0
    ohT = onehot.reshape(M, KK).T                        # (24, M)

    # 4-row packing: row group g handles m-tiles t = 4j + g
    oh4 = np.zeros((128, 8, 128), dtype=np.float32)
    for g in range(NBASE):
        for j in range(8):
            t = 4 * j + g
            oh4[32 * g:32 * g + KK, j, :] = ohT[:, t * 128:(t + 1) * 128]
    oh4 = np.ascontiguousarray(
        oh4.reshape(128, 8 * 128)).astype(ml_dtypes.bfloat16)

    if USE_FP8:
        freqT = np.ascontiguousarray(freq.T * 128.0).astype(
            ml_dtypes.float8_e4m3)
    else:
        freqT = np.ascontiguousarray(freq.T).astype(ml_dtypes.bfloat16)

    in_maps = []
    for c in range(NCORES):
        pfc = pf_flat[c * FL:(c + 1) * FL]               # (FL, 24)
        par4 = np.zeros((128, FL), dtype=np.float32)
        for g in range(NBASE):
            par4[32 * g:32 * g + KK, :] = pfc.T
        in_maps.append({
            "freqT": freqT,
            "oh4": oh4,
            "par4": np.ascontiguousarray(par4).astype(ml_dtypes.bfloat16),
        })
    return in_maps


def _run(in_maps, trace=False):
    nc = _get_nc()
    return run_bass_kernel_spmd(nc, in_maps, list(range(NCORES)), trace=trace)


def kernel(freq, kmer_params, temperature, kmer_idcs):
    in_maps = _prepare_in_maps(freq, kmer_params, temperature, kmer_idcs)
    res = _run(in_maps,
               trace=os.environ.get("KERNEL_TRACE", "") not in ("", "0"))
    _CACHE["last_result"] = res
    return np.concatenate(
        [np.asarray(res.results[c]["out"], dtype=np.float32)
         for c in range(NCORES)], axis=1)
